# revision 5
# baseline (speedup 1.0000x reference)
"""Causal multi-head self-attention (S=4096, D=1024, H=16, RoPE) on 8 trn2 cores.

Tensor-parallel over heads: core c owns heads 2c, 2c+1.
Pipeline per core:
  A) dma-transpose own x slice -> AllGather -> full xT [1024, 4096] bf16
  B) qT/kT projections in transposed+feature-grouped layout, RoPE, v natural
  C) flash-style causal attention with transposed scores; exp on ACT;
     denominator via ones-column in v; normalize -> headsT [128, 4096]
  D) AllGather headsT -> output projection for a 128-col slice of out
Host assembles out[0, :, 128c:128c+128] = outT_c.T
"""

import sys

for _p in ("/opt/trn_rl_repo", "/root/.axon_site/_ro/trn_rl_repo"):
    if _p not in sys.path:
        sys.path.append(_p)

import numpy as np
import ml_dtypes

import concourse.bass as bass
import concourse.tile as tile
from concourse import bacc, mybir
from concourse.bass_utils import run_bass_kernel_spmd

BF16 = mybir.dt.bfloat16
F32 = mybir.dt.float32
NPBF16 = ml_dtypes.bfloat16

S = 4096          # sequence
D = 1024          # model dim
NH = 16           # heads
DK = 64           # head dim
NCORES = 8
HPC = NH // NCORES          # 2 heads per core
P = HPC * DK                # 128 = per-core head feature count
SS = S // NCORES            # 512 = per-core seq slice
THETA = 10000.0
ST = 512                    # s-tile width (a-tile width too)
NT = S // ST                # 8 tiles
EXPFN = mybir.ActivationFunctionType.Exp

_CACHE = {}


def _build_program():
    nc = bacc.Bacc(
        "TRN2", target_bir_lowering=False, debug=False, num_devices=NCORES
    )

    # ---- I/O ----
    xs = nc.dram_tensor("xs", [SS, D], BF16, kind="ExternalInput").ap()
    wqT = nc.dram_tensor("wqT", [D, P], BF16, kind="ExternalInput").ap()
    wkT = nc.dram_tensor("wkT", [D, P], BF16, kind="ExternalInput").ap()
    wvT = nc.dram_tensor("wvT", [D, P], BF16, kind="ExternalInput").ap()
    woT = nc.dram_tensor("woT", [D, P], BF16, kind="ExternalInput").ap()
    cosg = nc.dram_tensor("cosg", [P, S], F32, kind="ExternalInput").ap()
    sing = nc.dram_tensor("sing", [P, S], F32, kind="ExternalInput").ap()
    masks = nc.dram_tensor("masks", [128, 4 * ST], BF16, kind="ExternalInput").ap()
    outT = nc.dram_tensor("outT", [P, S], F32, kind="ExternalOutput").ap()

    # ---- collective buffers ----
    cc_xt_in = nc.dram_tensor("cc_xt_in", [D, SS], BF16)
    cc_xt_out = nc.dram_tensor("cc_xt_out", [D * NCORES, SS], BF16, addr_space="Shared")
    cc_ho_in = nc.dram_tensor("cc_ho_in", [P, S], BF16)
    cc_ho_out = nc.dram_tensor("cc_ho_out", [P * NCORES, S], BF16, addr_space="Shared")

    rg = [list(range(NCORES))]

    with tile.TileContext(nc) as tc:
        with (
            tc.tile_pool(name="const", bufs=1) as constp,
            tc.tile_pool(name="big", bufs=1) as bigp,
            tc.tile_pool(name="xtp", bufs=3) as xtp,
            tc.tile_pool(name="xt", bufs=12) as xtld,
            tc.tile_pool(name="rope", bufs=3) as ropep,
            tc.tile_pool(name="pt", bufs=6) as ptp,
            tc.tile_pool(name="dinv", bufs=3) as dinvp,
            tc.tile_pool(name="hb", bufs=4) as hbp,
            tc.tile_pool(name="fout", bufs=3) as foutp,
            tc.tile_pool(name="psum", bufs=2, space="PSUM") as psp,
        ):
            # ---- constants ----
            cos_sb = constp.tile([P, S], F32, tag="cos")
            nc.sync.dma_start(cos_sb[:], cosg[:])
            sin_sb = constp.tile([P, S], F32, tag="sin")
            nc.sync.dma_start(sin_sb[:], sing[:])
            mask_sb = constp.tile([128, 4 * ST], BF16, tag="mask")
            nc.sync.dma_start(mask_sb[:], masks[:])

            def load_w(name, src):
                w = constp.tile([128, D], BF16, tag=name)
                for u in range(D // 128):
                    nc.sync.dma_start(
                        w[:, 128 * u : 128 * (u + 1)],
                        src[128 * u : 128 * (u + 1), :],
                    )
                return w

            wq_sb = load_w("wq", wqT)
            wk_sb = load_w("wk", wkT)
            wv_sb = load_w("wv", wvT)
            wo_sb = load_w("wo", woT)

            # ---- big persistent tiles ----
            q_sb = bigp.tile([P, S], BF16, tag="q")
            k_sb = bigp.tile([P, S], BF16, tag="k")
            # v chunks: per 128-row block B: cols [130B, 130B+130):
            #   h0 v at +0..63, h0 ones at +64, h1 v at +65..128, h1 ones at +129
            v_sb = bigp.tile([128, 130 * (S // 128)], BF16, tag="v")
            nc.vector.memset(v_sb[:], 1.0)
            ho0_sb = bigp.tile([DK, S], BF16, tag="ho0")
            ho1_sb = bigp.tile([DK, S], BF16, tag="ho1")

            # ---- phase A: transpose own x slice, allgather xT ----
            for k in range(D // 128):
                tt = xtp.tile([128, SS], BF16)
                nc.sync.dma_start_transpose(tt[:], xs[:, 128 * k : 128 * (k + 1)])
                nc.sync.dma_start(cc_xt_in.ap()[128 * k : 128 * (k + 1), :], tt[:])
            nc.gpsimd.collective_compute(
                "AllGather",
                mybir.AluOpType.bypass,
                ins=[cc_xt_in.ap()],
                outs=[cc_xt_out.ap()],
                replica_groups=rg,
            )

            # ---- phase B: projections + rope ----
            for t in range(NT):
                xts = []
                for u in range(D // 128):
                    xt_t = xtld.tile([128, ST], BF16)
                    base = D * t + 128 * u
                    nc.sync.dma_start(
                        xt_t[:], cc_xt_out.ap()[base : base + 128, :]
                    )
                    xts.append(xt_t)

                asl = slice(ST * t, ST * (t + 1))

                for (w_sb, dst) in ((wq_sb, q_sb), (wk_sb, k_sb)):
                    pp = psp.tile([128, ST], F32, tag="proj")
                    for u in range(8):
                        nc.tensor.matmul(
                            pp[:],
                            lhsT=w_sb[:, 128 * u : 128 * (u + 1)],
                            rhs=xts[u][:],
                            start=(u == 0),
                            stop=(u == 7),
                        )
                    # rope: dst = pp * cos + swap32(pp) * sin   (grouped layout)
                    pf = ropep.tile([128, ST], F32, tag="pf")
                    nc.vector.tensor_copy(pf[:], pp[:])
                    psw = ropep.tile([128, ST], F32, tag="psw")
                    for g in range(4):
                        src = (g ^ 1) * 32
                        nc.sync.dma_start(
                            psw[32 * g : 32 * (g + 1), :],
                            pf[src : src + 32, :],
                        )
                    m1 = ropep.tile([128, ST], F32, tag="m1")
                    nc.vector.tensor_mul(m1[:], pp[:], cos_sb[:, asl])
                    m2 = ropep.tile([128, ST], F32, tag="m2")
                    nc.vector.tensor_mul(m2[:], psw[:], sin_sb[:, asl])
                    nc.vector.tensor_add(dst[:, asl], m1[:], m2[:])

                for sx in range(4):
                    vp = psp.tile([128, 128], F32, tag="proj")
                    for u in range(8):
                        nc.tensor.matmul(
                            vp[:],
                            lhsT=xts[u][:, 128 * sx : 128 * (sx + 1)],
                            rhs=wv_sb[:, 128 * u : 128 * (u + 1)],
                            start=(u == 0),
                            stop=(u == 7),
                        )
                    B = 4 * t + sx
                    nc.vector.tensor_copy(v_sb[:, 130 * B : 130 * B + 64], vp[:, 0:64])
                    nc.vector.tensor_copy(
                        v_sb[:, 130 * B + 65 : 130 * B + 129], vp[:, 64:128]
                    )

            # ---- phase C: attention ----
            for A in range(NT):
                for h in range(2):
                    hsl = slice(DK * h, DK * (h + 1))
                    asl = slice(ST * A, ST * (A + 1))
                    op = psp.tile([65, ST], F32, tag="o")
                    nB = 4 * (A + 1)
                    for B in range(nB):
                        sp = psp.tile([128, ST], F32, tag="sc")
                        nc.tensor.matmul(
                            sp[:],
                            lhsT=k_sb[hsl, 128 * B : 128 * (B + 1)],
                            rhs=q_sb[hsl, asl],
                            start=True,
                            stop=True,
                        )
                        pt = ptp.tile([128, ST], BF16, tag="pt")
                        if B >= 4 * A:
                            j = B - 4 * A
                            pte = ptp.tile([128, ST], BF16, tag="pte")
                            nc.scalar.activation(pte[:], sp[:], EXPFN)
                            nc.vector.tensor_mul(
                                pt[:], pte[:], mask_sb[:, ST * j : ST * (j + 1)]
                            )
                        else:
                            nc.scalar.activation(pt[:], sp[:], EXPFN)
                        nc.tensor.matmul(
                            op[:],
                            lhsT=v_sb[:, 130 * B + 65 * h : 130 * B + 65 * h + 65],
                            rhs=pt[:],
                            start=(B == 0),
                            stop=(B == nB - 1),
                        )
                    dinv = dinvp.tile([1, ST], F32, tag="dinv")
                    nc.vector.reciprocal(dinv[:], op[64:65, :])
                    drep = dinvp.tile([DK, ST], F32, tag="drep")
                    nc.gpsimd.partition_broadcast(drep[:], dinv[:], channels=DK)
                    dst = ho0_sb if h == 0 else ho1_sb
                    nc.vector.tensor_mul(dst[:, asl], op[0:64, :], drep[:])

            nc.sync.dma_start(cc_ho_in.ap()[0:DK, :], ho0_sb[:])
            nc.sync.dma_start(cc_ho_in.ap()[DK:P, :], ho1_sb[:])
            nc.gpsimd.collective_compute(
                "AllGather",
                mybir.AluOpType.bypass,
                ins=[cc_ho_in.ap()],
                outs=[cc_ho_out.ap()],
                replica_groups=rg,
            )

            # ---- phase D: output projection (128-col slice of out) ----
            for t in range(NT):
                fp = psp.tile([128, ST], F32, tag="proj")
                for u in range(8):
                    hb = hbp.tile([128, ST], BF16)
                    nc.sync.dma_start(
                        hb[:],
                        cc_ho_out.ap()[128 * u : 128 * (u + 1), ST * t : ST * (t + 1)],
                    )
                    nc.tensor.matmul(
                        fp[:],
                        lhsT=wo_sb[:, 128 * u : 128 * (u + 1)],
                        rhs=hb[:],
                        start=(u == 0),
                        stop=(u == 7),
                    )
                fo = foutp.tile([128, ST], F32)
                nc.vector.tensor_copy(fo[:], fp[:])
                nc.sync.dma_start(outT[:, ST * t : ST * (t + 1)], fo[:])

    nc.compile()
    return nc


def _host_inputs(x, Wq, Wk, Wv, Wo):
    x2 = np.asarray(x).reshape(S, D)

    # grouped feature permutation per head: pos 64h+32o+f <- orig 64h+2f+o
    perm = np.empty(P, dtype=np.int64)
    for h in range(HPC):
        for o in range(2):
            for f in range(DK // 2):
                perm[DK * h + 32 * o + f] = DK * h + 2 * f + o

    pos = np.arange(S, dtype=np.float64)
    inv_freq = 1.0 / THETA ** (np.arange(0, DK, 2, dtype=np.float64) / DK)
    ang = np.outer(pos, inv_freq)  # [S, 32]
    cos32 = np.cos(ang).T.astype(np.float32)  # [32, S]
    sin32 = np.sin(ang).T.astype(np.float32)
    cosg = np.tile(cos32, (4, 1))  # [128, S] (same for E/O and both heads)
    sing = np.concatenate([-sin32, sin32, -sin32, sin32], axis=0)

    mk = np.zeros((128, 4 * ST), dtype=NPBF16)
    al = np.arange(ST)
    for j in range(4):
        bl = 128 * j + np.arange(128)
        mk[:, ST * j : ST * (j + 1)] = (bl[:, None] <= al[None, :]).astype(NPBF16)

    scale = 1.0 / np.sqrt(DK)
    in_maps = []
    for c in range(NCORES):
        rows = slice(P * c, P * (c + 1))
        wq_c = (np.asarray(Wq)[rows][perm] * scale).astype(np.float32)
        wk_c = np.asarray(Wk)[rows][perm].astype(np.float32)
        wv_c = np.asarray(Wv)[rows]
        wo_c = np.asarray(Wo)[rows]  # output rows 128c..128c+128, all input dims
        in_maps.append(
            {
                "xs": x2[SS * c : SS * (c + 1), :].astype(NPBF16),
                "wqT": np.ascontiguousarray(wq_c.T).astype(NPBF16),
                "wkT": np.ascontiguousarray(wk_c.T).astype(NPBF16),
                "wvT": np.ascontiguousarray(wv_c.T).astype(NPBF16),
                "woT": np.ascontiguousarray(wo_c.T).astype(NPBF16),
                "cosg": cosg,
                "sing": sing,
                "masks": mk,
            }
        )
    return in_maps


def get_program():
    if "nc" not in _CACHE:
        _CACHE["nc"] = _build_program()
    return _CACHE["nc"]


def kernel(x, Wq, Wk, Wv, Wo):
    nc = get_program()
    in_maps = _host_inputs(x, Wq, Wk, Wv, Wo)
    res = run_bass_kernel_spmd(nc, in_maps, list(range(NCORES)))
    out = np.empty((1, S, D), dtype=np.float32)
    for c in range(NCORES):
        out[0, :, P * c : P * (c + 1)] = res.results[c]["outT"].T
    return out


if __name__ == "__main__":
    import reference

    inputs = {k: np.asarray(v) for k, v in reference.setup_inputs().items()}
    got = kernel(**inputs)
    exp = np.asarray(reference.reference(**inputs))
    denom = np.abs(exp).max()
    err = np.abs(got - exp).max() / denom
    print(f"Relative error: {err:.3e}")
